# revision 59
# baseline (speedup 1.0000x reference)
"""Trainium2 Bass kernel: batched multi-head attention with per-frame
conditioning K/V token (nn_Attention dense_transformer problem).

Strategy: data-parallel over the 16 (b*n) frames -> 2 frames per NeuronCore,
no collectives. Per core, a fused kernel pipelined so the scalar engine's exp
stream (the softmax) rarely waits:

 - sim computed transposed (keys on partitions) so the PV matmul contracts
   over keys directly and softmax denominators come from a ones-column in
   the 65-wide PV stationary.
 - Both heads of a pair share one exp instruction: the QK row-tile pair
   writes sA/sB into adjacent PSUM banks of one [128, 2, 512] tile and a
   single activation covers 1024 elements/lane (halves ACT overhead).
 - The conditioning token is a 1-key mini-chunk (no 127 dummy keys):
   QK writes single sim rows at partitions 0/32 of one bank, exp covers
   [33, 512], and PV uses contraction-1 stationaries. No big memsets.
 - PSUM: 4 banks sim (2-deep QK->exp->PV pipeline) + 2 banks PV
   accumulators + 2 rotating work banks (proj chains / cond sim / bcast).
 - QKV/out projections run as background chains in fixed thunk slots
   inside the attention sections so the PE fills its slack while ACT runs.
 - Softmax 1/denom on the ACT engine as exp(-ln d) (Ln+Exp share one
   activation table set, pinned via _pin_act_tables so the table-load
   pass emits exactly one ACT_TABLE_LOAD) instead of the DVE's iterative
   reciprocal (8 cyc/elem, ~3.4us per section, was the critical path).
 - The whole normalization (ln/exp/broadcast/muls) is software-pipelined
   one section later, interleaved into the next section's QK/exp stream
   at fixed points so no engine FIFO ever stalls at a section boundary;
   each section also pre-emits the NEXT section's first QK+exp so the
   ACT queue flows straight across the boundary.
 - Cold-start trims: x rides the gpsimd DMA queue in parallel with
   weights on sync (first-needed-first); a tiny warm-up exp preloads the
   ACT table during the input DMA wait; the first k/q projection chains
   start as soon as their DMA chunks land.
 - bf16 output DMA (halves output traffic); host casts back to f32.
"""

import numpy as np
import ml_dtypes

import concourse.bacc as bacc
import concourse.tile as tile
from concourse import mybir
from concourse.bass_utils import run_bass_kernel_spmd

BF16 = mybir.dt.bfloat16
F32 = mybir.dt.float32

HEADS = 8
DH = 64
D = 512
HID = 512
SCALE = DH ** -0.5
N_CORES = 8
NDC = D // 128


INLINE_NORM = False

_ORIG_GET_TABLES = None


def _pin_act_tables():
    """Restrict Exp/Ln to the combined natural_log_exp_and_others set.

    The table-load insertion pass maps each activation to the first set
    containing its function, which alternates exp_and_others /
    natural_log and inserts a ~2.9us ACT_TABLE_LOAD per section. Both
    functions live in natural_log_exp_and_others (400-ULP tables), so
    removing them from every other set forces a single load. Set order
    (and thus act_func_set_id indices) is preserved.
    """
    global _ORIG_GET_TABLES
    import concourse.hw_specs as hw_specs
    if _ORIG_GET_TABLES is None:
        _ORIG_GET_TABLES = hw_specs.get_activation_tables

    def patched(arch):
        exp = mybir.ActivationFunctionType.Exp
        ln = mybir.ActivationFunctionType.Ln
        out = {}
        for name, fns in _ORIG_GET_TABLES(arch).items():
            if name != "natural_log_exp_and_others":
                fns = set(fns) - {exp, ln}
            out[name] = set(fns)
        return out

    bacc.get_activation_tables = patched


def build_attention_nc(T=1024, loop_n=1):
    _pin_act_tables()
    NI = 512
    NIH = T // NI
    NTC = T // 128
    KT_PAD = 1032

    nc = bacc.Bacc("TRN2", target_bir_lowering=False)
    x_d = nc.declare_dram_parameter("xT", [128, NDC, 2, T], BF16, isOutput=False)
    w_d = nc.declare_dram_parameter("Wqkv", [128, NDC, 3 * HID], BF16, isOutput=False)
    wk_d = nc.declare_dram_parameter("Wk", [128, NDC, HID], BF16, isOutput=False)
    wv_d = nc.declare_dram_parameter("Wv", [128, NDC, HID], BF16, isOutput=False)
    wo_d = nc.declare_dram_parameter("Wout", [128, NDC, D], BF16, isOutput=False)
    lab_d = nc.declare_dram_parameter("labT", [128, NDC, 2, 8], BF16, isOutput=False)
    f_d = nc.declare_dram_parameter("F", [33, 128], BF16, isOutput=False)
    out_d = nc.declare_dram_parameter("out", [2, T, D], BF16, isOutput=True)

    EXP = mybir.ActivationFunctionType.Exp
    LN = mybir.ActivationFunctionType.Ln

    with tile.TileContext(nc) as tc:
        with (
            tc.tile_pool(name="persist", bufs=1) as pp,
            tc.tile_pool(name="work", bufs=2) as wp,
            tc.tile_pool(name="psum", bufs=2, space="PSUM") as psp,
        ):
            def emit_body():
                xT = pp.tile([128, NDC, 2, T], BF16, tag="xT")
                wq = pp.tile([128, NDC, 3 * HID], BF16, tag="wq", bufs=2)
                wk = pp.tile([128, NDC, HID], BF16, tag="wk")
                wv = pp.tile([128, NDC, HID], BF16, tag="wv")
                wo = pp.tile([128, NDC, D], BF16, tag="wo")
                lab = pp.tile([128, NDC, 2, 8], BF16, tag="lab", bufs=2)
                qT = pp.tile([128, NDC, 2, T], BF16, tag="qT", bufs=2)
                kT = pp.tile([128, NDC, 2, KT_PAD], BF16, tag="kT", bufs=2)
                vv = pp.tile([128, 2, NTC, HEADS, 68], BF16, tag="vv", bufs=2)
                vcond = pp.tile([33, 2, HEADS, 68], BF16, tag="vcond", bufs=2)
                attn = pp.tile([128, NDC, 2, T], BF16, tag="attn", bufs=2)
                fmat = pp.tile([33, 128], BF16, tag="fmat", bufs=2)
                rg = pp.tile([33, NI], BF16, tag="rg", bufs=2)

                # Two parallel trigger queues. The hardware (sync) queue starts
                # ~2.7us in; the gpsimd SWDGE queue needs ~9us of Q7 startup —
                # so everything the first ~10us of compute touches goes on
                # sync, ordered first-needed-first, and only the frame-1 x
                # (needed ~60us in) rides the software queue.
                # f0-ih0 (the first K/Q chains' input) split across BOTH
                # queues so the chains aren't paced by one queue's transfer
                # rate: dc1/dc3 lead the gpsimd queue, dc0/dc2 ride sync right
                # after the two wq column blocks the same chains need.
                for dc in (1, 3):
                    nc.gpsimd.dma_start(xT[:, dc, 0, 0:T // 2], x_d[:, dc, 0, 0:T // 2])
                for dc in range(NDC):
                    nc.gpsimd.dma_start(xT[:, dc, 0, T // 2:T], x_d[:, dc, 0, T // 2:T])
                for dc in range(NDC):
                    nc.gpsimd.dma_start(xT[:, dc, 1], x_d[:, dc, 1])
                nc.sync.dma_start(wq[:, :, 512:640], w_d[:, :, 512:640])
                nc.sync.dma_start(wq[:, :, 0:128], w_d[:, :, 0:128])
                for dc in (0, 2):
                    nc.sync.dma_start(xT[:, dc, 0, 0:T // 2], x_d[:, dc, 0, 0:T // 2])
                nc.sync.dma_start(wq[:, :, 1024:1536], w_d[:, :, 1024:1536])
                nc.sync.dma_start(wk[:], wk_d[:])
                nc.sync.dma_start(lab[:], lab_d[:])
                nc.sync.dma_start(wv[:], wv_d[:])
                nc.sync.dma_start(fmat[:], f_d[:])
                for p in range(1, 4):
                    nc.sync.dma_start(
                        wq[:, :, 512 + p * 128:512 + (p + 1) * 128],
                        w_d[:, :, 512 + p * 128:512 + (p + 1) * 128])
                    nc.sync.dma_start(
                        wq[:, :, p * 128:(p + 1) * 128],
                        w_d[:, :, p * 128:(p + 1) * 128])
                nc.sync.dma_start(wo[:], wo_d[:])

                nc.vector.memset(rg[:], 1.0)
                warm = wp.tile([1, 8], BF16, tag="warm")
                nc.scalar.activation(warm[:], rg[0:1, 0:8], EXP, scale=SCALE)
                nc.vector.memset(vv[:, :, :, :, DH:DH + 1], 1.0)
                nc.vector.memset(vcond[0:1, :, :, DH:DH + 1], 1.0)
                nc.vector.memset(vcond[32:33, :, :, DH:DH + 1], 1.0)

                def emit_qk_proj(f, cc, ih):
                    ps = psp.tile([128, NI], F32, tag="work")
                    isl = slice(ih * NI, (ih + 1) * NI)
                    for dc in range(NDC):
                        nc.tensor.matmul(
                            ps[:],
                            wq[:, dc, cc * 128:(cc + 1) * 128],
                            xT[:, dc, f, isl],
                            start=(dc == 0), stop=(dc == NDC - 1),
                        )
                    if cc < 4:
                        nc.vector.tensor_copy(qT[:, cc, f, isl], ps[:])
                    else:
                        nc.vector.tensor_copy(kT[:, cc - 4, f, isl], ps[:])

                def emit_v(f, tc_i):
                    ps = psp.tile([128, HID], F32, tag="work")
                    for dc in range(NDC):
                        nc.tensor.matmul(
                            ps[:],
                            xT[:, dc, f, tc_i * 128:(tc_i + 1) * 128],
                            wq[:, dc, 2 * HID:3 * HID],
                            start=(dc == 0), stop=(dc == NDC - 1),
                        )
                    nc.vector.tensor_copy(vv[:, f, tc_i, :, 0:DH], ps[:])

                def emit_ek():
                    for cc in range(NDC):
                        ps = psp.tile([128, NI], F32, tag="work")
                        for dc in range(NDC):
                            nc.tensor.matmul(
                                ps[:, 0:2],
                                wk[:, dc, cc * 128:(cc + 1) * 128],
                                lab[:, dc, :, 0:1],
                                start=(dc == 0), stop=(dc == NDC - 1),
                            )
                        for f in range(2):
                            nc.vector.tensor_copy(kT[:, cc, f, T:T + 1], ps[:, f:f + 1])

                def emit_ev(f):
                    for base in (0, 32):
                        ps = psp.tile([128, HID], F32, tag="work")
                        for dc in range(NDC):
                            nc.tensor.matmul(
                                ps[base:base + 1, :],
                                lab[:, dc, f, 0:1],
                                wv[:, dc, :],
                                start=(dc == 0), stop=(dc == NDC - 1),
                            )
                        nc.vector.tensor_copy(
                            vcond[base:base + 1, f, :, 0:DH], ps[base:base + 1, :])

                def emit_outproj(f, tc_i):
                    ps = psp.tile([128, D], F32, tag="work")
                    for a in range(NDC):
                        nc.tensor.matmul(
                            ps[:],
                            attn[:, a, f, tc_i * 128:(tc_i + 1) * 128],
                            wo[:, a, :],
                            start=(a == 0), stop=(a == NDC - 1),
                        )
                    ot = wp.tile([128, D], BF16, tag="oout")
                    nc.vector.tensor_copy(ot[:], ps[:])
                    nc.sync.dma_start(out_d[f, tc_i * 128:(tc_i + 1) * 128, :], ot[:])

                # Single-buffered: the norm pipeline is strictly lag-1 and each
                # engine queue is in-order, so section s+1's writes naturally
                # wait behind section s's reads (WAR on the same queue).
                rgL = pp.tile([33, NI], F32, tag="rgL")
                rgE = pp.tile([33, NI], BF16, tag="rgE")
                # rows 1-31 are never written by the Ln ops but are read by the
                # full-tile exp; zero them so exp(-0)=1 lands there (harmless —
                # fmat rows 1-31 are zero) instead of exp(-garbage)=Inf -> NaN.
                nc.vector.memset(rgL[:], 0.0)

                def emit_qk_for(f2, a2, ih2, jc, sims):
                    simt = psp.tile([128, 2, NI], F32, tag="sim")
                    isl2 = slice(ih2 * NI, (ih2 + 1) * NI)
                    jsl = slice(jc * 128, (jc + 1) * 128)
                    nc.tensor.matmul(
                        simt[:, 0, :], kT[0:64, a2, f2, jsl], qT[0:64, a2, f2, isl2],
                        start=True, stop=True, tile_position=(0, 0),
                    )
                    nc.tensor.matmul(
                        simt[:, 1, :], kT[64:128, a2, f2, jsl], qT[64:128, a2, f2, isl2],
                        start=True, stop=True, tile_position=(64, 0),
                    )
                    sims.append(simt)

                def emit_exp_for(sims, Ps, jc):
                    P = wp.tile([128, 2, NI], BF16, tag="P", bufs=5)
                    nc.scalar.activation(P[:], sims[jc][:], EXP, scale=SCALE)
                    Ps.append(P)

                def emit_section(f, a, ih, thunks, prev, carried, nxt, sec_i):
                    isl = slice(ih * NI, (ih + 1) * NI)
                    pvA = psp.tile([65, NI], F32, tag="pv")
                    pvB = psp.tile([65, NI], F32, tag="pv")
                    ti = [0]

                    def pop_thunk():
                        if ti[0] < len(thunks):
                            thunks[ti[0]]()
                            ti[0] += 1

                    if carried is not None:
                        sims, Ps = carried
                    else:
                        sims, Ps = [], []

                    def emit_qk(jc):
                        emit_qk_for(f, a, ih, jc, sims)

                    def emit_exp(jc):
                        emit_exp_for(sims, Ps, jc)

                    def emit_pv(jc):
                        P = Ps[jc]
                        nc.tensor.matmul(
                            pvA[:], vv[:, f, jc, 2 * a, 0:65], P[:, 0, :],
                            start=(jc == 0), stop=False,
                        )
                        nc.tensor.matmul(
                            pvB[:], vv[:, f, jc, 2 * a + 1, 0:65], P[:, 1, :],
                            start=(jc == 0), stop=False,
                        )

                    # Software-pipelined normalization of the PREVIOUS section,
                    # interleaved so no engine queue ever stalls:
                    #  - prev["pre"] (ACT ln of denoms + DVE pv->SBUF copies) right
                    #    after this section's first exp, so the ACT ops sit behind
                    #    exp(c0) in the FIFO and their PV-stop dep is already met.
                    #  - prev["exp33"] (ACT 1/d = exp(-ln d)) one chunk later.
                    #  - prev["tail"] (PE denom broadcast + DVE muls) at chunk 3.
                    if carried is None:
                        emit_qk(0)
                        emit_exp(0)
                    if prev is not None:
                        prev["pre"]()
                    pop_thunk()
                    emit_qk(1)
                    emit_exp(1)
                    if prev is not None:
                        prev["exp33"]()
                    pop_thunk()
                    P8 = wp.tile([33, NI], BF16, tag="P8", bufs=1)
                    for jc in range(2, NTC):
                        emit_qk(jc)
                        emit_exp(jc)
                        # prev's tail MUST be emitted before this section's first
                        # PV matmul: the muls read prev's pv PSUM banks, which
                        # PV(0)/PV(1) re-start (same 2-buffer ring). Tile orders
                        # by emission, so tail-first is a correctness condition.
                        if jc == 2 and prev is not None:
                            prev["tail"]()
                        # cond-token sim + exp mid-section, so P8 is long done by
                        # the time the deferred cond-PV stops need it and the ACT
                        # queue never idles at the section boundary.
                        if jc == 5:
                            sim8 = psp.tile([33, NI], F32, tag="work")
                            nc.tensor.matmul(
                                sim8[0:1, :], kT[0:64, a, f, T:T + 1],
                                qT[0:64, a, f, isl],
                                start=True, stop=True, tile_position=(0, 0),
                            )
                            nc.tensor.matmul(
                                sim8[32:33, :], kT[64:128, a, f, T:T + 1],
                                qT[64:128, a, f, isl],
                                start=True, stop=True, tile_position=(64, 32),
                            )
                            nc.scalar.activation(P8[:], sim8[:], EXP, scale=SCALE)
                        emit_pv(jc - 2)
                        pop_thunk()
                    # pre-emit the NEXT section's chunk-0 QK+exp here, so the
                    # ACT queue flows straight from exp(7) into the next
                    # section's exp(0) while the PE runs this section's PV/cond
                    # tail — kills the ~2us ACT bubble at every boundary.
                    next_carried = None
                    # Pre-emit the next section's first TWO chunks so the ACT
                    # queue has ~2.2us of ready exps to ride out the PE's
                    # end-of-section backlog (PVs, cond stops, drain thunks).
                    # sec_i==0 excluded: (0,0,0)'s 13-thunk list drains its
                    # K(0,1,*)/Q(0,1,0) projections after this point, so the
                    # next section's QK would read unwritten kT/qT.
                    if nxt is not None and sec_i > 0:
                        nf, nih, na = nxt
                        nsims, nPs = [], []
                        emit_qk_for(nf, na, nih, 0, nsims)
                        emit_exp_for(nsims, nPs, 0)
                        next_carried = (nsims, nPs)
                    emit_pv(NTC - 2)
                    pop_thunk()
                    emit_pv(NTC - 1)
                    pop_thunk()
                    nc.tensor.matmul(
                        pvA[:], vcond[0:1, f, 2 * a, 0:65], P8[0:1, :],
                        start=False, stop=True,
                    )
                    nc.tensor.matmul(
                        pvB[:], vcond[32:33, f, 2 * a + 1, 0:65], P8[32:33, :],
                        start=False, stop=True,
                    )
                    while ti[0] < len(thunks):
                        pop_thunk()

                    def norm_pre():
                        nc.scalar.activation(rgL[0:1, :], pvA[64:65, :], LN)
                        nc.scalar.activation(rgL[32:33, :], pvB[64:65, :], LN)

                    def norm_exp33():
                        nc.scalar.activation(rgE[:], rgL[:], EXP, scale=-1.0)

                    def norm_tail():
                        bc = psp.tile([128, NI], F32, tag="work")
                        nc.tensor.matmul(bc[:], fmat[:], rgE[:], start=True, stop=True)
                        rbc = wp.tile([128, NI], BF16, tag="rbc", bufs=1)
                        nc.vector.tensor_copy(rbc[:], bc[:])
                        nc.vector.tensor_mul(attn[0:64, a, f, isl], pvA[0:64, :], rbc[0:64, :])
                        nc.vector.tensor_mul(attn[64:128, a, f, isl], pvB[0:64, :], rbc[64:128, :])

                    if INLINE_NORM:
                        norm_pre()
                        norm_exp33()
                        norm_tail()
                        return None, next_carried
                    return ({"pre": norm_pre, "exp33": norm_exp33,
                             "tail": norm_tail}, next_carried)

                K = lambda f, p, ih: (lambda: emit_qk_proj(f, 4 + p, ih))
                Q = lambda f, p, ih: (lambda: emit_qk_proj(f, p, ih))
                V = lambda f, t: (lambda: emit_v(f, t))
                EV = lambda f: (lambda: emit_ev(f))
                OP = lambda f, t: (lambda: emit_outproj(f, t))

                EKt = lambda: emit_ek()
                NOP = lambda: None
                emit_qk_proj(0, 4, 0)
                emit_qk_proj(0, 0, 0)
                emit_qk_proj(0, 4, 1)
                # first two V chains pre-loop: their inputs (wq V-columns +
                # x f0) land before the first section's QKs anyway, and
                # section (0,0,0) is otherwise PE-overloaded (13 thunks vs
                # ~4 chunks of slack), stalling the ACT exp stream.
                emit_v(0, 0)
                emit_v(0, 1)
                emit_v(0, 2)

                sched = {
                    # EKt must pop by chunk 4: the cond sim8 (emitted at
                    # chunk 5) reads the kT cond column EKt writes.
                    (0, 0, 0): [V(0, 3), EKt, V(0, 4), V(0, 5),
                                V(0, 6), V(0, 7), EV(0), K(0, 1, 0), K(0, 1, 1), Q(0, 1, 0)],
                    (0, 0, 1): [K(0, 2, 0), K(0, 2, 1), Q(0, 2, 0)],
                    (0, 0, 2): [K(0, 3, 0), K(0, 3, 1), Q(0, 3, 0)],
                    (0, 0, 3): [Q(0, 0, 1)],
                    (0, 1, 0): [Q(0, 1, 1), V(1, 0), V(1, 1)],
                    (0, 1, 1): [Q(0, 2, 1), V(1, 2), V(1, 3)],
                    (0, 1, 2): [Q(0, 3, 1), V(1, 4), V(1, 5)],
                    (0, 1, 3): [V(1, 6), V(1, 7), EV(1), K(1, 0, 0), K(1, 0, 1), Q(1, 0, 0)],
                    (1, 0, 0): [K(1, 1, 0), K(1, 1, 1), Q(1, 1, 0), Q(1, 0, 1), OP(0, 0)],
                    (1, 0, 1): [K(1, 2, 0), K(1, 2, 1), Q(1, 2, 0), Q(1, 1, 1), OP(0, 1)],
                    (1, 0, 2): [K(1, 3, 0), K(1, 3, 1), Q(1, 3, 0), Q(1, 2, 1), OP(0, 2)],
                    (1, 0, 3): [Q(1, 3, 1), OP(0, 3), OP(0, 4)],
                    # OP(1,0)/OP(1,1) need the muls of (1,0,3), which are
                    # emitted at this section's jc==2 (prev tail) — so they may
                    # only pop at thunk index >= 2 (chunk >= 2).
                    (1, 1, 0): [OP(0, 5), OP(0, 6), OP(1, 0), OP(1, 1)],
                    (1, 1, 1): [OP(0, 7), OP(1, 2), OP(1, 3)],
                }
                secs = [(f, ih, a) for f in range(2) for ih in range(NIH)
                        for a in range(NDC)]
                prev, carried = None, None
                for i, (f, ih, a) in enumerate(secs):
                    nxt = secs[i + 1] if i + 1 < len(secs) else None
                    prev, carried = emit_section(
                        f, a, ih, sched.get((f, ih, a), []), prev, carried, nxt, i)
                # Final four out-projections, a-split: accumulate their
                # a=0..2 partials into the now-free sim-ring PSUM banks while
                # the last section's softmax-norm chain runs on ACT/DVE; after
                # the final muls only the a=3 term + copy + DMA remain.
                acc0 = psp.tile([128, 2, NI], F32, tag="sim")
                acc1 = psp.tile([128, 2, NI], F32, tag="sim")
                accs = [acc0, acc1]
                for idx, t in enumerate(range(4, 8)):
                    acc = accs[idx // 2][:, idx % 2, :]
                    for a3 in range(3):
                        nc.tensor.matmul(
                            acc, attn[:, a3, 1, t * 128:(t + 1) * 128],
                            wo[:, a3, :], start=(a3 == 0), stop=False,
                        )
                if prev is not None:
                    prev["pre"]()
                    prev["exp33"]()
                    prev["tail"]()
                for idx, t in enumerate(range(4, 8)):
                    acc = accs[idx // 2][:, idx % 2, :]
                    nc.tensor.matmul(
                        acc, attn[:, 3, 1, t * 128:(t + 1) * 128],
                        wo[:, 3, :], start=False, stop=True,
                    )
                    ot = wp.tile([128, D], BF16, tag="oout")
                    # split the tail PSUM->SBUF copies across DVE and the
                    # now-idle ACT engine so they drain in parallel
                    if idx % 2 == 0:
                        nc.vector.tensor_copy(ot[:], acc)
                    else:
                        nc.scalar.activation(
                            ot[:], acc, mybir.ActivationFunctionType.Copy)
                    nc.sync.dma_start(out_d[1, t * 128:(t + 1) * 128, :], ot[:])

            if loop_n > 1 and loop_n % 2 == 0:
                with tc.For_i(0, loop_n // 2, 1):
                    emit_body()
                    emit_body()
            elif loop_n > 1:
                with tc.For_i(0, loop_n, 1):
                    emit_body()
            else:
                emit_body()

    nc.finalize()
    return nc


_NC_CACHE = {}


def _get_nc(T):
    if T not in _NC_CACHE:
        _NC_CACHE[T] = build_attention_nc(T)
    return _NC_CACHE[T]


def make_in_maps(x, label_emb_mm, Wqkv, Wk, Wv, Wout):
    bf = ml_dtypes.bfloat16
    BN, T, d = x.shape
    assert (BN, d) == (16, D)
    xB = np.ascontiguousarray(
        np.asarray(x).reshape(16, T, NDC, 128).transpose(0, 3, 2, 1)
    ).astype(bf)
    wq = np.ascontiguousarray(np.asarray(Wqkv).reshape(NDC, 128, 3 * HID).transpose(1, 0, 2)).astype(bf)
    wkh = np.ascontiguousarray(np.asarray(Wk).reshape(NDC, 128, HID).transpose(1, 0, 2)).astype(bf)
    wvh = np.ascontiguousarray(np.asarray(Wv).reshape(NDC, 128, HID).transpose(1, 0, 2)).astype(bf)
    woh = np.ascontiguousarray(np.asarray(Wout).reshape(NDC, 128, D).transpose(1, 0, 2)).astype(bf)
    labB = np.asarray(label_emb_mm).reshape(16, NDC, 128)
    F = np.zeros((33, 128), dtype=bf)
    F[0, 0:64] = 1.0
    F[32, 64:128] = 1.0
    in_maps = []
    for c in range(N_CORES):
        xTc = np.ascontiguousarray(xB[2 * c:2 * c + 2].transpose(1, 2, 0, 3))
        labc2 = np.ascontiguousarray(labB[2 * c:2 * c + 2].transpose(2, 1, 0)).astype(bf)
        labc = np.zeros((128, NDC, 2, 8), dtype=bf)
        labc[:, :, :, 0] = labc2
        in_maps.append({
            "xT": xTc, "Wqkv": wq, "Wk": wkh, "Wv": wvh, "Wout": woh, "labT": labc,
            "F": F,
        })
    return in_maps


def kernel(x, label_emb_mm, Wqkv, Wk, Wv, Wout, b):
    x = np.asarray(x)
    T = x.shape[1]
    nc = _get_nc(T)
    in_maps = make_in_maps(x, label_emb_mm, Wqkv, Wk, Wv, Wout)
    res = run_bass_kernel_spmd(nc, in_maps, core_ids=list(range(N_CORES)))
    out = np.concatenate([res.results[c]["out"] for c in range(N_CORES)], axis=0)
    return np.ascontiguousarray(out.reshape(16, T, D)).astype(np.float32)

